# revision 1
# baseline (speedup 1.0000x reference)
"""GAT (2-layer dense-graph attention over 4096 nodes) as a Trainium2
Bass/Tile SPMD kernel across 8 NeuronCores.

Sharding: attention destination rows are sharded 512/core for both layers.
Each core computes the full source-side quantities (h', d — tiny) from the
full x, and the s-scores only for its own 512 destination rows. The layer-0
output (transposed) is exchanged between layers with FOUR chunked
AllGathers (2 heads = 16 feature rows each) so gather latency overlaps the
remaining heads' attention compute.

Math notes (exactness): softmax_j(leakyrelu(s_i+d_j)) is invariant to any
per-row factor, so with E = exp(leakyrelu(z)) = max(e^z, e^{0.2 z}) we use
E' = E * e^{-0.2 s_i} = max(e^{0.8 s_i} e^{d_j}, e^{0.2 d_j}),
computed as ONE fused DVE tensor_scalar op per [128, 512] tile:
(a_tile * b_j) max c_j, with a = e^{0.8 s} replicated across partitions and
b = e^d, c = e^{0.2 d} as per-partition scalars. BatchNorm (eval mode) is
folded into the weights host-side.

Precision/perf: E is bf16 (single-pass PE matmuls instead of the fp32
LOW_HIGH double-pass; bf16 quantization of E largely cancels between the
softmax numerator and denominator). The aggregation values h' are kept at
~fp32 precision by splitting into bf16 high + bf16 residual parts placed at
partition-aligned stationary columns (0/32) with the softmax-denominator
ones-column at 64 — matmul cost is N-bound, so the extra columns are free.
Compute engines can only address partition bases 0/32/64/96, which dictates
those offsets; partition-shifted row assembly goes through sbuf->sbuf DMA.
"""

import numpy as np
import ml_dtypes

import concourse.bacc as bacc
import concourse.mybir as mybir
import concourse.tile as tile
from concourse import masks
from concourse.bass_utils import run_bass_kernel_spmd

F32 = mybir.dt.float32
BF16 = mybir.dt.bfloat16
N = 4096
NCORES = 8
RPC = N // NCORES          # destination rows per core = 512
NJT = N // 128             # 32 j-tiles of 128 source rows
BN_EPS = 1e-5

_CACHE = {}


def _build():
    nc = bacc.Bacc("TRN2", target_bir_lowering=False, debug=False,
                   num_devices=NCORES)

    x_d = nc.dram_tensor("x", [N, 32], F32, kind="ExternalInput")
    xs_d = nc.dram_tensor("x_slice", [RPC, 32], F32, kind="ExternalInput")
    w0all_d = nc.dram_tensor("w0all", [33, 80], F32, kind="ExternalInput")
    w0s_d = nc.dram_tensor("w0s", [33, 8], F32, kind="ExternalInput")
    w1all_d = nc.dram_tensor("w1all", [65, 33], F32, kind="ExternalInput")
    w1b_d = nc.dram_tensor("w1b", [1, 33], F32, kind="ExternalInput")
    w1sc_d = nc.dram_tensor("w1sc", [16, 4], F32, kind="ExternalInput")
    sb1_d = nc.dram_tensor("sb1t", [1, 1], F32, kind="ExternalInput")
    b0cc_d = nc.dram_tensor("b0cc", [16, 4], F32, kind="ExternalInput")
    b1_d = nc.dram_tensor("b1f", [32, 1], F32, kind="ExternalInput")
    sela_d = nc.dram_tensor("sela", [8, 8 * 128], BF16, kind="ExternalInput")
    s2sel_d = nc.dram_tensor("s2sel", [2, 16], F32, kind="ExternalInput")
    out_d = nc.dram_tensor("out", [RPC, 32], F32, kind="ExternalOutput")

    with tile.TileContext(nc) as tc:
        with (
            tc.tile_pool(name="const", bufs=1) as const,
            tc.tile_pool(name="persist", bufs=1) as per,
            tc.tile_pool(name="dram", bufs=1, space="DRAM") as dram,
        ):
            ident = const.tile([128, 128], F32)
            masks.make_identity(nc, ident[:])
            ones_row = const.tile([1, 128], F32)
            nc.vector.memset(ones_row[:], 1.0)
            ones_row_bf = const.tile([1, 128], BF16)
            nc.vector.memset(ones_row_bf[:], 1.0)
            ones512 = const.tile([1, 512], F32)
            nc.vector.memset(ones512[:], 1.0)
            sela = const.tile([8, 8 * 128], BF16)
            nc.sync.dma_start(sela[:], sela_d[:])
            s2sel = const.tile([2, 16], F32)
            nc.sync.dma_start(s2sel[:], s2sel_d[:])

            w0all = const.tile([33, 80], F32)
            nc.sync.dma_start(w0all[:], w0all_d[:])
            w0s = const.tile([33, 8], F32)
            nc.sync.dma_start(w0s[:], w0s_d[:])
            w1all = const.tile([65, 33], F32)
            nc.sync.dma_start(w1all[:], w1all_d[:])
            w1b = const.tile([1, 33], F32)
            nc.sync.dma_start(w1b[:], w1b_d[:])
            w1sc = const.tile([16, 4], F32)
            nc.sync.dma_start(w1sc[:], w1sc_d[:])
            sb1t = const.tile([1, 1], F32)
            nc.sync.dma_start(sb1t[:], sb1_d[:])
            b0cc = const.tile([16, 4], F32)
            nc.sync.dma_start(b0cc[:], b0cc_d[:])
            b1c = const.tile([32, 1], F32)
            nc.sync.dma_start(b1c[:], b1_d[:])

            # big persistent sbuf tensors
            xT = per.tile([33, N], F32)        # x^T plus ones row
            xsT = per.tile([33, RPC], F32)     # x_slice^T plus ones row
            # stationary operand per (jt, h): hi(0:8) res(32:40) ones(64)
            hpa0 = per.tile([128, NJT, 8, 66], BF16)
            d0e = per.tile([128, NJT, 8], F32)       # e^{d0}
            d0e2 = per.tile([128, NJT, 8], F32)      # e^{0.2 d0}
            atile = per.tile([128, 8, 512], BF16)    # e^{0.8 s0} bcast
            outTNc = per.tile([16, 4, 512], F32)     # L0 numerators^T/chunk
            rowsc = per.tile([2, 4, 512], F32)       # L0 denominators/chunk
            contc = per.tile([16, 4, 512], F32)      # elu(out0)^T per chunk
            hTag = per.tile([65, 8, 512], F32)       # gathered h^T blocks
            # stationary per jt: hi(0:32) res(32:64) ones(64)
            hpa1 = per.tile([128, NJT, 66], BF16)
            d1e = per.tile([128, NJT], F32)
            d1e2 = per.tile([128, NJT], F32)
            a1tile = per.tile([128, 512], BF16)
            a0row = per.tile([8, 512], BF16)
            a1row = per.tile([1, 512], BF16)
            r1row = per.tile([1, 512], F32)
            num1 = per.tile([32, 512], F32)
            res1s = per.tile([32, 512], F32)
            norm1 = per.tile([32, 512], F32)

            contd = [dram.tile([16, 512], F32, name=f"contd{c}",
                               tag=f"contd{c}") for c in range(4)]
            agc = [dram.tile([NCORES * 16, 512], F32, name=f"agc{c}",
                             tag=f"agc{c}") for c in range(4)]

            # ---------------- Phase A: projections -----------------
            with (
                tc.tile_pool(name="ld", bufs=2) as ld,
                tc.tile_pool(name="tp", bufs=2, space="PSUM") as tp,
                tc.tile_pool(name="mm80", bufs=2, space="PSUM") as mm80,
                tc.tile_pool(name="pssa0", bufs=1, space="PSUM") as pssa0,
                tc.tile_pool(name="pssa", bufs=2, space="PSUM") as pssa,
                tc.tile_pool(name="wp", bufs=1, space="PSUM") as wp,
            ):
                # PE warm-up burst: ~20 back-to-back matmuls flip the HAM
                # clock gate to 8/8 while input DMAs are still in flight
                wsrc = ld.tile([128, 512], BF16, tag="wsrc")
                nc.vector.memset(wsrc[:], 0.5)
                wlhs = ld.tile([128, 128], BF16, tag="wlhs")
                nc.vector.memset(wlhs[:], 0.25)
                wps = wp.tile([128, 512], F32)
                for r in range(20):
                    nc.tensor.matmul(wps[:], wlhs[:], wsrc[:],
                                     start=(r == 0), stop=(r == 19))
                # x -> xT (32 transposes), x_slice -> xsT (4 transposes)
                xbig = ld.tile([128, NJT, 32], F32, tag="xbig")
                nc.sync.dma_start(
                    xbig[:], x_d[:].rearrange("(k p) c -> p k c", p=128))
                for k in range(NJT):
                    pt = tp.tile([32, 128], F32)
                    nc.tensor.matmul(pt[:], xbig[:, k, :], ident[:, :],
                                     is_transpose=True)
                    nc.vector.tensor_copy(xT[0:32, k * 128:(k + 1) * 128],
                                          pt[:])
                nc.vector.memset(xT[32:33, :], 1.0)

                xsbig = ld.tile([128, 4, 32], F32, tag="xsbig")
                nc.sync.dma_start(
                    xsbig[:], xs_d[:].rearrange("(k p) c -> p k c", p=128))
                for k in range(4):
                    pt = tp.tile([32, 128], F32)
                    nc.tensor.matmul(pt[:], xsbig[:, k, :], ident[:, :],
                                     is_transpose=True)
                    nc.vector.tensor_copy(xsT[0:32, k * 128:(k + 1) * 128],
                                          pt[:])
                nc.vector.memset(xsT[32:33, :], 1.0)

                # s0 rows for this core's 512 dst rows; a = e^{0.8 s}
                ps0 = pssa0.tile([8, 512], F32, tag="ps0")
                nc.tensor.matmul(ps0[:], w0s[:], xsT[:])
                nc.scalar.activation(a0row[:], ps0[:],
                                     mybir.ActivationFunctionType.Exp,
                                     scale=0.8)
                for h in range(8):
                    pa = pssa.tile([128, 512], F32, tag="pa")
                    nc.tensor.matmul(pa[:], sela[:, h * 128:(h + 1) * 128],
                                     a0row[:])
                    nc.vector.tensor_copy(atile[:, h, :], pa[:])

                # h'0 (hi+res), d0 exps per j-tile
                nc.vector.memset(hpa0[:], 0.0)
                nc.vector.memset(hpa0[:, :, :, 64:65], 1.0)
                for jt in range(NJT):
                    p80 = mm80.tile([128, 80], F32)
                    nc.tensor.matmul(p80[:], xT[:, jt * 128:(jt + 1) * 128],
                                     w0all[:])
                    hsrc = p80[:, 0:64].rearrange("p (h o) -> p h o", h=8)
                    nc.vector.tensor_copy(hpa0[:, jt, :, 0:8], hsrc)
                    # residual = fp32 h' - bf16(h')
                    nc.vector.tensor_tensor(hpa0[:, jt, :, 32:40], hsrc,
                                            hpa0[:, jt, :, 0:8],
                                            op=mybir.AluOpType.subtract)
                    nc.scalar.activation(d0e[:, jt, :], p80[:, 64:72],
                                         mybir.ActivationFunctionType.Exp)
                    nc.scalar.activation(d0e2[:, jt, :], p80[:, 64:72],
                                         mybir.ActivationFunctionType.Exp,
                                         scale=0.2)

            # ------- Phase B/C: layer-0 attention, chunked gather -------
            with (
                tc.tile_pool(name="epool", bufs=10) as epool,
                tc.tile_pool(name="agg", bufs=3, space="PSUM") as agg,
                tc.tile_pool(name="rb", bufs=2, space="PSUM") as rb,
                tc.tile_pool(name="tmp", bufs=2) as tmp,
            ):
                for h in range(8):
                    ch, hh = h // 2, h % 2
                    pg = agg.tile([65, 512], F32)
                    for jt in range(NJT):
                        e = epool.tile([128, 512], BF16, tag="e")
                        nc.vector.tensor_scalar(
                            e[:], atile[:, h, :],
                            d0e[:, jt, h:h + 1], d0e2[:, jt, h:h + 1],
                            op0=mybir.AluOpType.mult,
                            op1=mybir.AluOpType.max)
                        nc.tensor.matmul(pg[:], hpa0[:, jt, h, 0:65], e[:],
                                         start=(jt == 0), stop=(jt == NJT - 1))
                    # hi + residual numerators; engines address base 0/32/64
                    stgr = tmp.tile([8, 512], F32, tag="stgr")
                    nc.vector.tensor_copy(stgr[:], pg[32:40, :])
                    stgn = tmp.tile([8, 512], F32, tag="stgn")
                    nc.vector.tensor_tensor(stgn[:], pg[0:8, :], stgr[:],
                                            op=mybir.AluOpType.add)
                    stgd = tmp.tile([1, 512], F32, tag="stgd")
                    nc.vector.tensor_copy(stgd[:], pg[64:65, :])
                    nc.sync.dma_start(outTNc[hh * 8:(hh + 1) * 8, ch, :],
                                      stgn[:])
                    nc.sync.dma_start(rowsc[hh:hh + 1, ch, :], stgd[:])

                    if hh == 1:
                        # chunk ch complete: normalize + bias + ELU + gather
                        rrc = tmp.tile([2, 512], F32, tag="rrc")
                        nc.vector.reciprocal(rrc[:], rowsc[:, ch, :])
                        prb = rb.tile([16, 512], F32)
                        nc.tensor.matmul(prb[:], s2sel[:], rrc[:])
                        nrm = tmp.tile([16, 512], F32, tag="nrm")
                        nc.vector.tensor_tensor(nrm[:], outTNc[:, ch, :],
                                                prb[:],
                                                op=mybir.AluOpType.mult)
                        nc.vector.tensor_scalar_add(nrm[:], nrm[:],
                                                    b0cc[:, ch:ch + 1])
                        mneg = tmp.tile([16, 512], F32, tag="mneg")
                        nc.vector.tensor_scalar_min(mneg[:], nrm[:], 0.0)
                        eneg = tmp.tile([16, 512], F32, tag="eneg")
                        nc.scalar.activation(
                            eneg[:], mneg[:],
                            mybir.ActivationFunctionType.Exp)
                        ppos = tmp.tile([16, 512], F32, tag="ppos")
                        nc.vector.tensor_scalar_max(ppos[:], nrm[:], 0.0)
                        # elu = (eneg - 1) + ppos
                        nc.vector.scalar_tensor_tensor(
                            contc[:, ch, :], eneg[:], -1.0, ppos[:],
                            op0=mybir.AluOpType.add,
                            op1=mybir.AluOpType.add)
                        nc.sync.dma_start(contd[ch][:], contc[:, ch, :])
                        nc.gpsimd.collective_compute(
                            "AllGather",
                            mybir.AluOpType.bypass,
                            replica_groups=[list(range(NCORES))],
                            ins=[contd[ch].opt()],
                            outs=[agc[ch].opt()],
                        )
                        nc.sync.dma_start(
                            hTag[ch * 16:(ch + 1) * 16, :, :],
                            agc[ch][:].rearrange("(b r) f -> r b f", r=16))

                nc.vector.memset(hTag[64:65, :, :], 1.0)

            # ---------------- Phase D: layer 1 ----------------
            with (
                tc.tile_pool(name="e1pool", bufs=6) as e1pool,
                tc.tile_pool(name="mmd", bufs=2, space="PSUM") as mmd,
                tc.tile_pool(name="pd", bufs=1, space="PSUM") as pd,
                tc.tile_pool(name="agg1", bufs=1, space="PSUM") as agg1,
                tc.tile_pool(name="tp2", bufs=2, space="PSUM") as tp2,
                tc.tile_pool(name="ot", bufs=2) as ot,
            ):
                # s1 from the local contribution chunks (+ ones * sb1)
                ps1 = pd.tile([1, 512], F32, tag="ps1")
                for c in range(4):
                    nc.tensor.matmul(ps1[:], w1sc[:, c:c + 1],
                                     contc[:, c, :],
                                     start=(c == 0), stop=False)
                nc.tensor.matmul(ps1[:], sb1t[:], ones512[:],
                                 start=False, stop=True)
                nc.scalar.activation(a1row[:], ps1[:],
                                     mybir.ActivationFunctionType.Exp,
                                     scale=0.8)
                pa1 = pd.tile([128, 512], F32, tag="pa1")
                nc.tensor.matmul(pa1[:], ones_row_bf[:], a1row[:])
                nc.vector.tensor_copy(a1tile[:], pa1[:])

                nc.vector.memset(hpa1[:, :, 64:65], 1.0)
                for jt in range(NJT):
                    blk, kk = jt // 4, jt % 4
                    p34 = mmd.tile([128, 33], F32)
                    nc.tensor.matmul(
                        p34[:], hTag[:, blk, kk * 128:(kk + 1) * 128],
                        w1all[:])
                    nc.vector.tensor_copy(hpa1[:, jt, 0:32], p34[:, 0:32])
                    nc.vector.tensor_tensor(hpa1[:, jt, 32:64], p34[:, 0:32],
                                            hpa1[:, jt, 0:32],
                                            op=mybir.AluOpType.subtract)
                    nc.scalar.activation(d1e[:, jt:jt + 1], p34[:, 32:33],
                                         mybir.ActivationFunctionType.Exp)
                    nc.scalar.activation(d1e2[:, jt:jt + 1], p34[:, 32:33],
                                         mybir.ActivationFunctionType.Exp,
                                         scale=0.2)

                pg1 = agg1.tile([65, 512], F32)
                for jt in range(NJT):
                    e1 = e1pool.tile([128, 512], BF16, tag="e1")
                    nc.vector.tensor_scalar(
                        e1[:], a1tile[:],
                        d1e[:, jt:jt + 1], d1e2[:, jt:jt + 1],
                        op0=mybir.AluOpType.mult,
                        op1=mybir.AluOpType.max)
                    nc.tensor.matmul(pg1[:], hpa1[:, jt, 0:65], e1[:],
                                     start=(jt == 0), stop=(jt == NJT - 1))

                nc.vector.reciprocal(r1row[:], pg1[64:65, :])
                prb1 = pd.tile([32, 512], F32, tag="prb1")
                nc.tensor.matmul(prb1[:], ones_row[0:1, 0:32], r1row[:])
                nc.vector.tensor_copy(res1s[:], pg1[32:64, :])
                nc.vector.tensor_tensor(num1[:], pg1[0:32, :], res1s[:],
                                        op=mybir.AluOpType.add)
                nc.vector.tensor_tensor(norm1[:], num1[:], prb1[:],
                                        op=mybir.AluOpType.mult)
                nc.vector.tensor_scalar_add(norm1[:], norm1[:], b1c[:])

                for ic in range(4):
                    pt2 = tp2.tile([128, 32], F32)
                    nc.tensor.matmul(pt2[:],
                                     norm1[:, ic * 128:(ic + 1) * 128],
                                     ident[0:32, 0:32], is_transpose=True)
                    ob = ot.tile([128, 32], F32, tag="ob")
                    nc.vector.tensor_copy(ob[:], pt2[:])
                    nc.sync.dma_start(out_d[ic * 128:(ic + 1) * 128, :],
                                      ob[:])

    nc.compile()
    return nc


def _fold(inputs):
    """Host-side BN folding and attention-projection folding (numpy)."""
    f64 = np.float64
    x = np.ascontiguousarray(np.asarray(inputs["x"], np.float32))
    w0 = np.asarray(inputs["w0"], f64)          # [8, 32, 8]
    w1 = np.asarray(inputs["w1"], f64)          # [1, 64, 32]
    a_src0 = np.asarray(inputs["a_src0"], f64)[..., 0]   # [8, 8]
    a_dst0 = np.asarray(inputs["a_dst0"], f64)[..., 0]   # [8, 8]
    a_src1 = np.asarray(inputs["a_src1"], f64)[0, :, 0]  # [32]
    a_dst1 = np.asarray(inputs["a_dst1"], f64)[0, :, 0]  # [32]
    b0 = np.asarray(inputs["b0"], f64)          # [8]
    b1 = np.asarray(inputs["b1"], f64)          # [32]

    al0 = np.asarray(inputs["bn0_gamma"], f64) / np.sqrt(
        np.asarray(inputs["bn0_var"], f64) + BN_EPS)
    sh0 = np.asarray(inputs["bn0_beta"], f64) - \
        np.asarray(inputs["bn0_mean"], f64) * al0
    al1 = np.asarray(inputs["bn1_gamma"], f64) / np.sqrt(
        np.asarray(inputs["bn1_var"], f64) + BN_EPS)
    sh1 = np.asarray(inputs["bn1_beta"], f64) - \
        np.asarray(inputs["bn1_mean"], f64) * al1

    # layer 0 folds
    w0flat = (al0[None, :, None] * w0).transpose(1, 0, 2).reshape(32, 64)
    beta0h = np.einsum("i,hio->ho", sh0, w0)     # [8, 8]
    beta0 = beta0h.reshape(64)
    as0 = al0[:, None] * np.einsum("hio,ho->ih", w0, a_src0)   # [32, 8]
    sb0 = np.einsum("ho,ho->h", beta0h, a_src0)
    ad0 = al0[:, None] * np.einsum("hio,ho->ih", w0, a_dst0)
    db0 = np.einsum("ho,ho->h", beta0h, a_dst0)

    w0all = np.zeros((33, 80), f64)
    w0all[0:32, 0:64] = w0flat
    w0all[32, 0:64] = beta0
    w0all[0:32, 64:72] = ad0
    w0all[32, 64:72] = db0
    w0s = np.zeros((33, 8), f64)
    w0s[0:32, :] = as0
    w0s[32, :] = sb0

    # layer 1 folds
    w1m = w1[0]                                   # [64, 32]
    w1flat = al1[:, None] * w1m
    beta1 = sh1 @ w1m                             # [32]
    as1 = al1 * (w1m @ a_src1)
    sb1 = beta1 @ a_src1
    ad1 = al1 * (w1m @ a_dst1)
    db1 = beta1 @ a_dst1

    w1all = np.zeros((65, 33), f64)
    w1all[0:64, 0:32] = w1flat
    w1all[64, 0:32] = beta1
    w1all[0:64, 32] = ad1
    w1all[64, 32] = db1

    b0f = np.tile(b0, 8)                          # (h,o) flat -> b0[o]
    b0cc = b0f.reshape(4, 16).T                   # [16, 4] per chunk
    b1f = b1.reshape(32, 1)
    w1sc = as1.reshape(4, 16).T                   # [16, 4] per chunk
    sb1t = np.array([[sb1]])

    sela = np.zeros((8, 8, 128), ml_dtypes.bfloat16)  # row h ones in block h
    for h in range(8):
        sela[h, h, :] = 1.0
    s2sel = np.zeros((2, 16), np.float32)         # S[p, m] = (m//8 == p)
    for p in range(2):
        s2sel[p, p * 8:(p + 1) * 8] = 1.0

    return {
        "x": x,
        "w0all": w0all.astype(np.float32),
        "w0s": w0s.astype(np.float32),
        "w1all": w1all.astype(np.float32),
        "w1b": w1all[64:65, :].astype(np.float32),
        "w1sc": w1sc.astype(np.float32),
        "sb1t": sb1t.astype(np.float32),
        "b0cc": b0cc.astype(np.float32),
        "b1f": b1f.astype(np.float32),
        "sela": sela.reshape(8, 8 * 128),
        "s2sel": s2sel,
    }


def kernel(**inputs) -> np.ndarray:
    if "nc" not in _CACHE:
        _CACHE["nc"] = _build()
    nc = _CACHE["nc"]

    shared = _fold(inputs)
    x = shared["x"]
    in_maps = []
    for c in range(NCORES):
        m = dict(shared)
        m["x_slice"] = np.ascontiguousarray(x[c * RPC:(c + 1) * RPC])
        in_maps.append(m)

    res = run_bass_kernel_spmd(nc, in_maps, list(range(NCORES)))
    out = np.concatenate([res.results[c]["out"] for c in range(NCORES)],
                         axis=0)
    return out.astype(np.float32)



# revision 19
# speedup vs baseline: 1.1974x; 1.1974x over previous
"""GAT (2-layer dense-graph attention over 4096 nodes) as a Trainium2
Bass/Tile SPMD kernel across 8 NeuronCores.

v3 structure:
- Layer 0 is DST-sharded (each core owns 512 destination rows, full
  4096-source h'/d computed per core). Layer 1 is SOURCE-sharded: each
  core's own 512 layer-0 output rows are exactly its layer-1 source
  nodes, so no inter-layer feature gather is needed. Each core computes
  layer-1 partial numerators/denominators for ALL 4096 destinations
  over its 512 sources; one ReduceScatter(add) then hands every core
  the summed numerators for its own 512 rows.
- Only collectives: a tiny AllGather of s1 (512 floats/core) and the
  final ReduceScatter (bf16). A dummy 32B AllGather fires at kernel
  start to absorb the device barrier + CC warmup off the critical path.
- E' = max(e^{0.8 s_i} e^{d_j}, e^{0.2 d_j}) (exact leakyrelu-softmax
  rescale) as ONE DVE tensor_scalar per [128, 512] tile, bf16.
- No fp32-residual columns: stationaries are bf16 h' (+ones col at 32
  for the denominator). All biases/BN folded host-side into beta rows.
- elu computed as contp = elu(x)+1 = max(x,0) + min(e^x, 1) (one
  ScalarE exp + 2 DVE ops); the -1 is folded into layer-1 beta rows.
- Reciprocals via exp(-ln(x)) on ScalarE: keeps every activation in
  the natural_log_exp_and_others table set (zero table reloads).
- d0 scores accumulate into a persistent PSUM bank; two big batched
  ScalarE exps produce all 256 per-(jt,h) scalars.
- Dummy matmuls keep the PE HAM clock-gate warm across the s1-gather
  wait so layer 1 runs at 2.4 GHz.
"""

import numpy as np
import ml_dtypes

import concourse.bacc as bacc
import concourse.mybir as mybir
import concourse.tile as tile
from concourse import masks
from concourse.bass_utils import run_bass_kernel_spmd

F32 = mybir.dt.float32
BF16 = mybir.dt.bfloat16
AF = mybir.ActivationFunctionType
OP = mybir.AluOpType
N = 4096
NCORES = 8
RPC = N // NCORES          # rows per core = 512
NJT = N // 128             # 32 j-tiles of 128 source rows
BN_EPS = 1e-5

_CACHE = {}


def _build():
    nc = bacc.Bacc("TRN2", target_bir_lowering=False, debug=False,
                   num_devices=NCORES)

    x_d = nc.dram_tensor("x", [N, 32], F32, kind="ExternalInput")
    xs_d = nc.dram_tensor("x_slice", [RPC, 32], F32, kind="ExternalInput")
    w0all_d = nc.dram_tensor("w0all", [33, 72], F32, kind="ExternalInput")
    w0s_d = nc.dram_tensor("w0s", [33, 8], F32, kind="ExternalInput")
    w1ext_d = nc.dram_tensor("w1ext", [65, 65], F32, kind="ExternalInput")
    sela_d = nc.dram_tensor("sela", [8, 8 * 128], BF16, kind="ExternalInput")
    s2sel_d = nc.dram_tensor("s2sel", [2, 16], F32, kind="ExternalInput")
    out_d = nc.dram_tensor("out", [RPC, 32], F32, kind="ExternalOutput")

    with tile.TileContext(nc) as tc:
        with (
            tc.tile_pool(name="const", bufs=1) as const,
            tc.tile_pool(name="per", bufs=1) as per,
            tc.tile_pool(name="psper", bufs=1, space="PSUM") as psper,
            tc.tile_pool(name="dram", bufs=1, space="DRAM") as dram,
        ):
            # ---------- dram intermediates ----------
            dum_i = dram.tile([1, 8], F32, name="dum_i", tag="dum_i")
            dum_o = dram.tile([8, 8], F32, name="dum_o", tag="dum_o")
            s1d = dram.tile([1, RPC], F32, name="s1d", tag="s1d")
            s1g = dram.tile([NCORES, RPC], F32, name="s1g", tag="s1g")
            rsin = dram.tile([NCORES, 33, RPC], BF16, name="rsin", tag="rsin")
            rsout = dram.tile([33, RPC], BF16, name="rsout", tag="rsout")

            # dummy collective first: absorbs device barrier + CC warmup
            nc.gpsimd.collective_compute(
                "AllGather", OP.bypass,
                replica_groups=[list(range(NCORES))],
                ins=[dum_i.opt()], outs=[dum_o.opt()])

            # ---------- consts ----------
            ident = const.tile([128, 128], F32)
            masks.make_identity(nc, ident[:])
            ones512 = const.tile([1, RPC], F32)
            nc.vector.memset(ones512[:], 1.0)
            ones32 = const.tile([1, 32], F32)
            nc.vector.memset(ones32[:], 1.0)
            sela = const.tile([8, 8 * 128], BF16)
            nc.sync.dma_start(sela[:], sela_d[:])
            s2sel = const.tile([2, 16], F32)
            nc.sync.dma_start(s2sel[:], s2sel_d[:])
            w0all = const.tile([33, 72], F32)
            nc.sync.dma_start(w0all[:], w0all_d[:])
            w0s = const.tile([33, 8], F32)
            nc.sync.dma_start(w0s[:], w0s_d[:])
            # layer-1 stationaries: per-chunk [16, 65] + beta row [1, 65]
            w1c = [const.tile([16, 65], F32, name=f"w1c{c}", tag=f"w1c{c}")
                   for c in range(4)]
            for c in range(4):
                nc.sync.dma_start(w1c[c][:], w1ext_d[16 * c:16 * c + 16, :])
            w1last = const.tile([1, 65], F32)
            nc.sync.dma_start(w1last[:], w1ext_d[64:65, :])

            # ---------- persistent sbuf ----------
            xT = per.tile([33, N], F32)
            xsT = per.tile([33, RPC], F32)
            hpa0 = per.tile([128, NJT, 8, 34], BF16)   # h' 0:8, ones @32
            d0e = per.tile([128, NJT * 8], F32)        # e^{d0}
            d0e2 = per.tile([128, NJT * 8], F32)       # e^{0.2 d0}
            atile = per.tile([128, 8, RPC], BF16)      # e^{0.8 s0} bcast
            nums = per.tile([16, 4, RPC], F32)         # L0 numerators/chunk
            dens = per.tile([2, 4, RPC], F32)          # L0 denominators/chunk
            contp = per.tile([16, 4, RPC], F32)        # elu(out0)+1 chunks
            hp1s = per.tile([33, RPC], F32)            # h'1 local (+d1 row 32)
            hpa1 = per.tile([128, 4, 34], BF16)        # L1 stationary
            d1e = per.tile([128, 4], F32)
            d1e2 = per.tile([128, 4], F32)
            s1s = per.tile([1, RPC], F32)
            s1raw = per.tile([8, RPC], F32)
            a1g = per.tile([8, RPC], BF16)
            a1t = per.tile([128, 8, RPC], BF16)
            num32 = per.tile([32, RPC], F32)
            outv = per.tile([32, RPC], F32)
            rso = per.tile([33, RPC], BF16)

            # persistent psum: s1/h'1 accumulator + L0 d-score bank
            ps1 = psper.tile([65, RPC], F32)
            dbank = psper.tile([128, NJT * 8], F32)

            # ---------------- Phase A: warmup + prep ----------------
            with (
                tc.tile_pool(name="ld", bufs=2) as ld,
                tc.tile_pool(name="tp", bufs=2, space="PSUM") as tp,
                tc.tile_pool(name="mm64", bufs=1, space="PSUM") as mm64,
                tc.tile_pool(name="ps0p", bufs=1, space="PSUM") as ps0p,
                tc.tile_pool(name="pab", bufs=2, space="PSUM") as pab,
            ):
                # PE warm-up burst (HAM clock gate -> 8/8); shares the
                # bcast pool tag so no extra PSUM bank is consumed
                wsrc = ld.tile([128, 512], BF16, tag="wsrc")
                nc.vector.memset(wsrc[:], 0.5)
                wlhs = ld.tile([128, 128], BF16, tag="wlhs")
                nc.vector.memset(wlhs[:], 0.25)
                wps = pab.tile([128, RPC], F32, tag="pa")
                for r in range(20):
                    nc.tensor.matmul(wps[:], wlhs[:], wsrc[:],
                                     start=(r == 0), stop=(r == 19))

                # x_slice -> xsT (4 transposes) for this core's dst rows
                xsbig = ld.tile([128, 4, 32], F32, tag="xsbig")
                nc.sync.dma_start(
                    xsbig[:], xs_d[:].rearrange("(k p) c -> p k c", p=128))
                for k in range(4):
                    pt = tp.tile([32, 128], F32)
                    nc.tensor.matmul(pt[:], xsbig[:, k, :], ident[:, :],
                                     is_transpose=True)
                    nc.vector.tensor_copy(xsT[0:32, k * 128:(k + 1) * 128],
                                          pt[:])
                nc.vector.memset(xsT[32:33, :], 1.0)

                # s0 for own 512 dst rows; atile = e^{0.8 s0} bcast
                ps0 = ps0p.tile([8, RPC], F32, tag="ps0")
                nc.tensor.matmul(ps0[:], w0s[:], xsT[:])
                a0row = ld.tile([8, RPC], BF16, tag="a0row")
                nc.scalar.activation(a0row[:], ps0[:], AF.Exp, scale=0.8)
                for h in range(8):
                    pa = pab.tile([128, RPC], F32, tag="pa")
                    nc.tensor.matmul(pa[:], sela[:, h * 128:(h + 1) * 128],
                                     a0row[:])
                    nc.scalar.copy(atile[:, h, :], pa[:])

                # x -> xT (32 transposes); h'0 + d0 per j-tile
                xbig = ld.tile([128, NJT, 32], F32, tag="xbig")
                nc.sync.dma_start(
                    xbig[:], x_d[:].rearrange("(k p) c -> p k c", p=128))
                nc.vector.memset(hpa0[:], 0.0)
                nc.vector.memset(hpa0[:, :, :, 32:33], 1.0)
                for k in range(NJT):
                    pt = tp.tile([32, 128], F32)
                    nc.tensor.matmul(pt[:], xbig[:, k, :], ident[:, :],
                                     is_transpose=True)
                    nc.vector.tensor_copy(xT[0:32, k * 128:(k + 1) * 128],
                                          pt[:])
                nc.vector.memset(xT[32:33, :], 1.0)
                for jt in range(NJT):
                    p64 = mm64.tile([128, 64], F32, tag="p64")
                    nc.tensor.matmul(p64[:], xT[:, jt * 128:(jt + 1) * 128],
                                     w0all[:, 0:64])
                    nc.tensor.matmul(dbank[:, jt * 8:(jt + 1) * 8],
                                     xT[:, jt * 128:(jt + 1) * 128],
                                     w0all[:, 64:72])
                    nc.vector.tensor_copy(
                        hpa0[:, jt, :, 0:8],
                        p64[:].rearrange("p (h o) -> p h o", h=8))
                # batched exps for all (jt, h) d-scalars
                nc.scalar.activation(d0e[:], dbank[:], AF.Exp)
                nc.scalar.activation(d0e2[:], dbank[:], AF.Exp, scale=0.2)

            # ---------------- Phase B: layer-0 attention ----------------
            with (
                tc.tile_pool(name="epool", bufs=8) as epool,
                tc.tile_pool(name="agg", bufs=3, space="PSUM") as agg,
                tc.tile_pool(name="rb", bufs=1, space="PSUM") as rb,
                tc.tile_pool(name="tmp", bufs=2) as tmp,
            ):
                for h in range(8):
                    ch, hh = h // 2, h % 2
                    pg = agg.tile([33, RPC], F32)
                    for jt in range(NJT):
                        e = epool.tile([128, RPC], BF16, tag="e")
                        nc.vector.tensor_scalar(
                            e[:], atile[:, h, :],
                            d0e[:, jt * 8 + h:jt * 8 + h + 1],
                            d0e2[:, jt * 8 + h:jt * 8 + h + 1],
                            op0=OP.mult, op1=OP.max)
                        nc.tensor.matmul(pg[:], hpa0[:, jt, h, 0:33], e[:],
                                         start=(jt == 0), stop=(jt == NJT - 1))
                    # evacuate numerators + denominator (ScalarE), assemble
                    # chunk tiles via SBUF->SBUF DMA (partition-base free)
                    stg = tmp.tile([8, RPC], F32, tag="stg")
                    nc.scalar.copy(stg[:], pg[0:8, :])
                    std = tmp.tile([1, RPC], F32, tag="std")
                    nc.scalar.copy(std[:], pg[32:33, :])
                    nc.sync.dma_start(nums[hh * 8:(hh + 1) * 8, ch, :], stg[:])
                    nc.sync.dma_start(dens[hh:hh + 1, ch, :], std[:])

                    if hh == 1:
                        # chunk complete: normalize + (elu+1) -> contp
                        lnd = tmp.tile([2, RPC], F32, tag="lnd")
                        nc.scalar.activation(lnd[:], dens[:, ch, :], AF.Ln)
                        rcp = tmp.tile([2, RPC], F32, tag="rcp")
                        nc.scalar.activation(rcp[:], lnd[:], AF.Exp,
                                             scale=-1.0)
                        prbc = rb.tile([16, RPC], F32)
                        nc.tensor.matmul(prbc[:], s2sel[:], rcp[:])
                        nrm = tmp.tile([16, RPC], F32, tag="nrm")
                        nc.vector.tensor_tensor(nrm[:], nums[:, ch, :],
                                                prbc[:], op=OP.mult)
                        texp = tmp.tile([16, RPC], F32, tag="texp")
                        nc.scalar.activation(texp[:], nrm[:], AF.Exp)
                        t1 = tmp.tile([16, RPC], F32, tag="t1")
                        nc.vector.tensor_scalar_min(t1[:], texp[:], 1.0)
                        nc.vector.scalar_tensor_tensor(
                            contp[:, ch, :], nrm[:], 0.0, t1[:],
                            op0=OP.max, op1=OP.add)
                        # accumulate s1 / h'1 / d1 (all in ps1)
                        nc.tensor.matmul(ps1[:], w1c[ch][:], contp[:, ch, :],
                                         start=(ch == 0), stop=False)
                        if ch == 3:
                            nc.tensor.matmul(ps1[:], w1last[:], ones512[:],
                                             start=False, stop=True)

            # ---------------- Phase C: inter-layer + layer 1 ----------------
            with (
                tc.tile_pool(name="ld2", bufs=2) as ld2,
                tc.tile_pool(name="tp2", bufs=2, space="PSUM") as tp2,
                tc.tile_pool(name="pa1p", bufs=2, space="PSUM") as pa1p,
                tc.tile_pool(name="agg1", bufs=2, space="PSUM") as agg1,
                tc.tile_pool(name="e1pool", bufs=6) as e1pool,
                tc.tile_pool(name="rssp", bufs=2) as rssp,
                tc.tile_pool(name="otp", bufs=2) as otp,
            ):
                # s1 out the door first: evac -> DRAM -> AllGather
                nc.scalar.copy(s1s[:], ps1[64:65, :])
                nc.sync.dma_start(s1d[:], s1s[:])
                nc.gpsimd.collective_compute(
                    "AllGather", OP.bypass,
                    replica_groups=[list(range(NCORES))],
                    ins=[s1d.opt()], outs=[s1g.opt()])
                nc.sync.dma_start(s1raw[:], s1g[:])

                # local h'1 -> transposed bf16 stationary + d1 exps
                nc.scalar.copy(hp1s[0:32, :], ps1[0:32, :])
                nc.scalar.copy(hp1s[32:33, :], ps1[32:33, :])
                for q in range(4):
                    ptq = tp2.tile([128, 33], F32, tag="ptq")
                    nc.tensor.matmul(ptq[:],
                                     hp1s[:, q * 128:(q + 1) * 128],
                                     ident[0:33, 0:33], is_transpose=True)
                    nc.vector.tensor_copy(hpa1[:, q, 0:32], ptq[:, 0:32])
                    nc.scalar.activation(d1e[:, q:q + 1], ptq[:, 32:33],
                                         AF.Exp)
                    nc.scalar.activation(d1e2[:, q:q + 1], ptq[:, 32:33],
                                         AF.Exp, scale=0.2)
                nc.vector.memset(hpa1[:, :, 32:33], 1.0)
                nc.vector.memset(hpa1[:, :, 33:34], 0.0)

                # keep PE hot while the s1 AllGather is in flight
                wsrc2 = ld2.tile([128, 128], BF16, tag="wsrc2")
                nc.vector.memset(wsrc2[:], 0.5)
                wlhs2 = ld2.tile([128, 128], BF16, tag="wlhs2")
                nc.vector.memset(wlhs2[:], 0.25)
                wps2 = tp2.tile([128, 33], F32, tag="ptq")
                for r in range(60):
                    nc.tensor.matmul(wps2[:], wlhs2[:], wsrc2[:, 0:33],
                                     start=(r == 0), stop=(r == 59))

                # gathered s1 -> a1 = e^{0.8 s1} (bf16), bcast per dst chunk
                nc.scalar.activation(a1g[:], s1raw[:], AF.Exp, scale=0.8)

                for ic in range(8):
                    pa1 = pa1p.tile([128, RPC], F32)
                    nc.tensor.matmul(pa1[:], sela[:, ic * 128:(ic + 1) * 128],
                                     a1g[:])
                    nc.scalar.copy(a1t[:, ic, :], pa1[:])
                    pg1 = agg1.tile([33, RPC], F32)
                    for jt in range(4):
                        e1 = e1pool.tile([128, RPC], BF16, tag="e1")
                        nc.vector.tensor_scalar(
                            e1[:], a1t[:, ic, :],
                            d1e[:, jt:jt + 1], d1e2[:, jt:jt + 1],
                            op0=OP.mult, op1=OP.max)
                        nc.tensor.matmul(pg1[:], hpa1[:, jt, 0:33], e1[:],
                                         start=(jt == 0), stop=(jt == 3))
                    rss = rssp.tile([33, RPC], BF16, tag="rss")
                    nc.scalar.copy(rss[0:32, :], pg1[0:32, :])
                    nc.scalar.copy(rss[32:33, :], pg1[32:33, :])
                    nc.sync.dma_start(rsin[ic, :, :], rss[:])

                # sum partials across cores; receive own 512 dst rows
                nc.gpsimd.collective_compute(
                    "ReduceScatter", OP.add,
                    replica_groups=[list(range(NCORES))],
                    ins=[rsin.opt()], outs=[rsout.opt()])
                nc.sync.dma_start(rso[:], rsout[:])

                # normalize + write out
                lnd1 = ld2.tile([1, RPC], F32, tag="lnd1")
                nc.scalar.activation(lnd1[:], rso[32:33, :], AF.Ln)
                rcp1 = ld2.tile([1, RPC], F32, tag="rcp1")
                nc.scalar.activation(rcp1[:], lnd1[:], AF.Exp, scale=-1.0)
                prb1 = agg1.tile([33, RPC], F32, tag="pg1")
                nc.tensor.matmul(prb1[0:32, :], ones32[:], rcp1[:])
                nc.scalar.copy(num32[:], rso[0:32, :])
                nc.vector.tensor_tensor(outv[:], num32[:], prb1[0:32, :],
                                        op=OP.mult)
                for q in range(4):
                    pt2 = tp2.tile([128, 33], F32, tag="ptq")
                    nc.tensor.matmul(pt2[:, 0:32],
                                     outv[:, q * 128:(q + 1) * 128],
                                     ident[0:32, 0:32], is_transpose=True)
                    ob = otp.tile([128, 32], F32, tag="ob")
                    nc.vector.tensor_copy(ob[:], pt2[:, 0:32])
                    nc.sync.dma_start(out_d[q * 128:(q + 1) * 128, :], ob[:])

    nc.compile()
    return nc


def _fold(inputs):
    """Host-side BN/bias/attention-projection folding (numpy, f64)."""
    f64 = np.float64
    x = np.ascontiguousarray(np.asarray(inputs["x"], np.float32))
    w0 = np.asarray(inputs["w0"], f64)          # [8, 32, 8]
    w1 = np.asarray(inputs["w1"], f64)          # [1, 64, 32]
    a_src0 = np.asarray(inputs["a_src0"], f64)[..., 0]   # [8, 8]
    a_dst0 = np.asarray(inputs["a_dst0"], f64)[..., 0]   # [8, 8]
    a_src1 = np.asarray(inputs["a_src1"], f64)[0, :, 0]  # [32]
    a_dst1 = np.asarray(inputs["a_dst1"], f64)[0, :, 0]  # [32]
    b0 = np.asarray(inputs["b0"], f64)          # [8]
    b1 = np.asarray(inputs["b1"], f64)          # [32]

    al0 = np.asarray(inputs["bn0_gamma"], f64) / np.sqrt(
        np.asarray(inputs["bn0_var"], f64) + BN_EPS)
    sh0 = np.asarray(inputs["bn0_beta"], f64) - \
        np.asarray(inputs["bn0_mean"], f64) * al0
    al1 = np.asarray(inputs["bn1_gamma"], f64) / np.sqrt(
        np.asarray(inputs["bn1_var"], f64) + BN_EPS)
    sh1 = np.asarray(inputs["bn1_beta"], f64) - \
        np.asarray(inputs["bn1_mean"], f64) * al1

    # layer 0 folds (BN into projection; b0 into the beta row so the
    # aggregation numerator directly includes the output bias)
    w0flat = (al0[None, :, None] * w0).transpose(1, 0, 2).reshape(32, 64)
    beta0h = np.einsum("i,hio->ho", sh0, w0)     # [8, 8]
    beta0 = (beta0h + b0[None, :]).reshape(64)
    as0 = al0[:, None] * np.einsum("hio,ho->ih", w0, a_src0)   # [32, 8]
    sb0 = np.einsum("ho,ho->h", beta0h, a_src0)
    ad0 = al0[:, None] * np.einsum("hio,ho->ih", w0, a_dst0)
    db0 = np.einsum("ho,ho->h", beta0h, a_dst0)

    w0all = np.zeros((33, 72), f64)
    w0all[0:32, 0:64] = w0flat
    w0all[32, 0:64] = beta0
    w0all[0:32, 64:72] = ad0
    w0all[32, 64:72] = db0
    w0s = np.zeros((33, 8), f64)
    w0s[0:32, :] = as0
    w0s[32, :] = sb0

    # layer 1 folds; input arrives as contp = elu(out0)+1, so each
    # output column's beta absorbs -sum_f(weights) plus BN/bias terms
    w1m = w1[0]                                   # [64, 32]
    w1flat = al1[:, None] * w1m                   # [64, 32]
    beta1 = sh1 @ w1m + b1                        # [32]
    as1 = al1 * (w1m @ a_src1)                    # [64]
    sb1 = (sh1 @ w1m) @ a_src1
    ad1 = al1 * (w1m @ a_dst1)
    db1 = (sh1 @ w1m) @ a_dst1

    w1ext = np.zeros((65, 65), f64)
    w1ext[0:64, 0:32] = w1flat
    w1ext[64, 0:32] = beta1 - w1flat.sum(axis=0)
    w1ext[0:64, 32] = ad1
    w1ext[64, 32] = db1 - ad1.sum()
    w1ext[0:64, 64] = as1
    w1ext[64, 64] = sb1 - as1.sum()

    sela = np.zeros((8, 8, 128), ml_dtypes.bfloat16)  # row h ones in block h
    for h in range(8):
        sela[h, h, :] = 1.0
    s2sel = np.zeros((2, 16), np.float32)         # S[p, m] = (m//8 == p)
    for p in range(2):
        s2sel[p, p * 8:(p + 1) * 8] = 1.0

    return {
        "x": x,
        "w0all": w0all.astype(np.float32),
        "w0s": w0s.astype(np.float32),
        "w1ext": w1ext.astype(np.float32),
        "sela": sela.reshape(8, 8 * 128),
        "s2sel": s2sel,
    }


def kernel(**inputs) -> np.ndarray:
    if "nc" not in _CACHE:
        _CACHE["nc"] = _build()
    nc = _CACHE["nc"]

    shared = _fold(inputs)
    x = shared["x"]
    in_maps = []
    for c in range(NCORES):
        m = dict(shared)
        m["x_slice"] = np.ascontiguousarray(x[c * RPC:(c + 1) * RPC])
        in_maps.append(m)

    res = run_bass_kernel_spmd(nc, in_maps, list(range(NCORES)))
    out = np.concatenate([res.results[c]["out"] for c in range(NCORES)],
                         axis=0)
    return out.astype(np.float32)


# revision 22
# speedup vs baseline: 1.3260x; 1.1074x over previous
"""GAT (2-layer dense-graph attention over 4096 nodes) as a Trainium2
Bass/Tile SPMD kernel across 8 NeuronCores.

Structure:
- Layer 0 DST-sharded (512 destination rows/core, full 4096-source
  h'/d per core). Layer 1 SOURCE-sharded: each core's own 512 layer-0
  output rows are its layer-1 sources; partial numerators/denominators
  for ALL 4096 destinations are summed with one bf16 ReduceScatter.
- Collectives: tiny s1 AllGather (2KB/core) + final ReduceScatter. A
  dummy AllGather at kernel start absorbs the device barrier + CC
  warmup off the critical path.
- x arrives host-transposed (xT [32,4096]) so all DMAs are contiguous
  16KB-per-partition reads - no on-chip transposes of x, no strided
  gather DMA.
- E' = max(e^{0.8 s_i} e^{d_j}, e^{0.2 d_j}) (exact leakyrelu-softmax
  rescale) as ONE DVE tensor_scalar per tile, bf16.
- elu as contp = elu(x)+1 = max(x,0) + min(e^x,1); the -1 folded into
  layer-1 beta rows. All biases/BN folded host-side.
- Reciprocals via DVE reciprocal_approx_fast (~18 bits, one op):
  ScalarE uses only Exp/Copy -> a single ACT table set, zero reloads.
- d0 scores accumulate into a persistent PSUM bank; two batched
  ScalarE exps produce all 256 per-(jt,h) softmax scalars.
- Dummy matmuls keep the PE HAM clock-gate warm across the s1-gather
  and ReduceScatter waits.
"""

import numpy as np
import ml_dtypes

import concourse.bacc as bacc
import concourse.mybir as mybir
import concourse.tile as tile
from concourse import masks
from concourse.bass_utils import run_bass_kernel_spmd

F32 = mybir.dt.float32
BF16 = mybir.dt.bfloat16
AF = mybir.ActivationFunctionType
OP = mybir.AluOpType
N = 4096
NCORES = 8
RPC = N // NCORES          # rows per core = 512
NJT = N // 128             # 32 j-tiles of 128 source rows
BN_EPS = 1e-5

_CACHE = {}


def _build():
    nc = bacc.Bacc("TRN2", target_bir_lowering=False, debug=False,
                   num_devices=NCORES)

    xT_d = nc.dram_tensor("xT", [32, N], F32, kind="ExternalInput")
    xsT_d = nc.dram_tensor("xsT", [32, RPC], F32, kind="ExternalInput")
    w0all_d = nc.dram_tensor("w0all", [33, 72], F32, kind="ExternalInput")
    w0s_d = nc.dram_tensor("w0s", [33, 8], F32, kind="ExternalInput")
    w1ext_d = nc.dram_tensor("w1ext", [65, 65], F32, kind="ExternalInput")
    sela_d = nc.dram_tensor("sela", [8, 8 * 128], BF16, kind="ExternalInput")
    s2sel_d = nc.dram_tensor("s2sel", [2, 16], F32, kind="ExternalInput")
    out_d = nc.dram_tensor("out", [RPC, 32], F32, kind="ExternalOutput")

    with tile.TileContext(nc) as tc:
        with (
            tc.tile_pool(name="const", bufs=1) as const,
            tc.tile_pool(name="per", bufs=1) as per,
            tc.tile_pool(name="psper", bufs=1, space="PSUM") as psper,
            tc.tile_pool(name="dram", bufs=1, space="DRAM") as dram,
        ):
            # ---------- dram intermediates ----------
            dum_i = dram.tile([1, 8], F32, name="dum_i", tag="dum_i")
            dum_o = dram.tile([8, 8], F32, name="dum_o", tag="dum_o")
            s1d = dram.tile([1, RPC], F32, name="s1d", tag="s1d")
            s1g = dram.tile([NCORES, RPC], F32, name="s1g", tag="s1g")
            rsin = dram.tile([NCORES, 33, RPC], BF16, name="rsin", tag="rsin")
            rsout = dram.tile([33, RPC], BF16, name="rsout", tag="rsout")

            # dummy collective first: absorbs device barrier + CC warmup
            nc.gpsimd.collective_compute(
                "AllGather", OP.bypass,
                replica_groups=[list(range(NCORES))],
                ins=[dum_i.opt()], outs=[dum_o.opt()])

            # ---------- consts ----------
            ident = const.tile([128, 128], F32)
            masks.make_identity(nc, ident[:])
            ones512 = const.tile([1, RPC], F32)
            nc.vector.memset(ones512[:], 1.0)
            ones32 = const.tile([1, 32], F32)
            nc.vector.memset(ones32[:], 1.0)
            sela = const.tile([8, 8 * 128], BF16)
            nc.sync.dma_start(sela[:], sela_d[:])
            s2sel = const.tile([2, 16], F32)
            nc.sync.dma_start(s2sel[:], s2sel_d[:])
            w0all = const.tile([33, 72], F32)
            nc.sync.dma_start(w0all[:], w0all_d[:])
            w0s = const.tile([33, 8], F32)
            nc.sync.dma_start(w0s[:], w0s_d[:])
            w1c = [const.tile([16, 65], F32, name=f"w1c{c}", tag=f"w1c{c}")
                   for c in range(4)]
            for c in range(4):
                nc.sync.dma_start(w1c[c][:], w1ext_d[16 * c:16 * c + 16, :])
            w1last = const.tile([1, 65], F32)
            nc.sync.dma_start(w1last[:], w1ext_d[64:65, :])

            # ---------- persistent sbuf ----------
            xT = per.tile([33, N], F32)
            xsT = per.tile([33, RPC], F32)
            hpa0 = per.tile([128, NJT, 8, 34], BF16)   # h' 0:8, ones @32
            d0e = per.tile([128, NJT * 8], F32)        # e^{d0}
            d0e2 = per.tile([128, NJT * 8], F32)       # e^{0.2 d0}
            atile = per.tile([128, 8, RPC], BF16)      # e^{0.8 s0} bcast
            nums = per.tile([16, 4, RPC], F32)
            dens = per.tile([2, 4, RPC], F32)
            contp = per.tile([16, 4, RPC], F32)        # elu(out0)+1 chunks
            hp1s = per.tile([33, RPC], F32)
            hpa1 = per.tile([128, 4, 34], BF16)
            d1e = per.tile([128, 4], F32)
            d1e2 = per.tile([128, 4], F32)
            s1s = per.tile([1, RPC], F32)
            s1raw = per.tile([8, RPC], F32)
            a1g = per.tile([8, RPC], BF16)
            a1t = per.tile([128, 8, RPC], BF16)
            num32 = per.tile([32, RPC], F32)
            outv = per.tile([32, RPC], F32)
            rso = per.tile([33, RPC], BF16)
            rcp1 = per.tile([1, RPC], F32)

            ps1 = psper.tile([65, RPC], F32)
            dbank = psper.tile([128, NJT * 8], F32)

            # ---------------- Phase A: warmup + prep ----------------
            with (
                tc.tile_pool(name="ld", bufs=2) as ld,
                tc.tile_pool(name="mm64", bufs=2, space="PSUM") as mm64,
                tc.tile_pool(name="ps0p", bufs=1, space="PSUM") as ps0p,
                tc.tile_pool(name="pab", bufs=2, space="PSUM") as pab,
            ):
                wsrc = ld.tile([128, 512], BF16, tag="wsrc")
                nc.vector.memset(wsrc[:], 0.5)
                wlhs = ld.tile([128, 128], BF16, tag="wlhs")
                nc.vector.memset(wlhs[:], 0.25)
                wps = pab.tile([128, RPC], F32, tag="pa")
                for r in range(24):
                    nc.tensor.matmul(wps[:], wlhs[:], wsrc[:],
                                     start=(r == 0), stop=(r == 23))

                # host-transposed inputs: contiguous big-granule DMAs
                nc.sync.dma_start(xT[0:32, :], xT_d[:])
                nc.vector.memset(xT[32:33, :], 1.0)
                nc.sync.dma_start(xsT[0:32, :], xsT_d[:])
                nc.vector.memset(xsT[32:33, :], 1.0)

                # s0 for own 512 dst rows; atile = e^{0.8 s0} bcast
                ps0 = ps0p.tile([8, RPC], F32, tag="ps0")
                nc.tensor.matmul(ps0[:], w0s[:], xsT[:])
                a0row = ld.tile([8, RPC], BF16, tag="a0row")
                nc.scalar.activation(a0row[:], ps0[:], AF.Exp, scale=0.8)
                for h in range(8):
                    pa = pab.tile([128, RPC], F32, tag="pa")
                    nc.tensor.matmul(pa[:], sela[:, h * 128:(h + 1) * 128],
                                     a0row[:])
                    nc.scalar.copy(atile[:, h, :], pa[:])

                # h'0 + d0 per j-tile
                nc.vector.memset(hpa0[:], 0.0)
                nc.vector.memset(hpa0[:, :, :, 32:33], 1.0)
                for jt in range(NJT):
                    p64 = mm64.tile([128, 64], F32, tag="p64")
                    nc.tensor.matmul(p64[:], xT[:, jt * 128:(jt + 1) * 128],
                                     w0all[:, 0:64])
                    nc.tensor.matmul(dbank[:, jt * 8:(jt + 1) * 8],
                                     xT[:, jt * 128:(jt + 1) * 128],
                                     w0all[:, 64:72])
                    nc.vector.tensor_copy(
                        hpa0[:, jt, :, 0:8],
                        p64[:].rearrange("p (h o) -> p h o", h=8))
                nc.scalar.activation(d0e[:], dbank[:], AF.Exp)
                nc.scalar.activation(d0e2[:], dbank[:], AF.Exp, scale=0.2)

            # ---------------- Phase B: layer-0 attention ----------------
            with (
                tc.tile_pool(name="epool", bufs=8) as epool,
                tc.tile_pool(name="agg", bufs=3, space="PSUM") as agg,
                tc.tile_pool(name="rb", bufs=1, space="PSUM") as rb,
                tc.tile_pool(name="tmp", bufs=2) as tmp,
            ):
                for h in range(8):
                    ch, hh = h // 2, h % 2
                    pg = agg.tile([33, RPC], F32)
                    for jt in range(NJT):
                        e = epool.tile([128, RPC], BF16, tag="e")
                        nc.vector.tensor_scalar(
                            e[:], atile[:, h, :],
                            d0e[:, jt * 8 + h:jt * 8 + h + 1],
                            d0e2[:, jt * 8 + h:jt * 8 + h + 1],
                            op0=OP.mult, op1=OP.max)
                        nc.tensor.matmul(pg[:], hpa0[:, jt, h, 0:33], e[:],
                                         start=(jt == 0), stop=(jt == NJT - 1))
                    stg = tmp.tile([8, RPC], F32, tag="stg")
                    nc.scalar.copy(stg[:], pg[0:8, :])
                    std = tmp.tile([1, RPC], F32, tag="std")
                    nc.scalar.copy(std[:], pg[32:33, :])
                    nc.sync.dma_start(nums[hh * 8:(hh + 1) * 8, ch, :], stg[:])
                    nc.sync.dma_start(dens[hh:hh + 1, ch, :], std[:])

                    if hh == 1:
                        # chunk complete: normalize + (elu+1) -> contp
                        rcp = tmp.tile([2, RPC], F32, tag="rcp")
                        nc.vector.reciprocal_approx_fast(rcp[:], dens[:, ch, :])
                        prbc = rb.tile([16, RPC], F32)
                        nc.tensor.matmul(prbc[:], s2sel[:], rcp[:])
                        nrm = tmp.tile([16, RPC], F32, tag="nrm")
                        nc.vector.tensor_tensor(nrm[:], nums[:, ch, :],
                                                prbc[:], op=OP.mult)
                        texp = tmp.tile([16, RPC], F32, tag="texp")
                        nc.scalar.activation(texp[:], nrm[:], AF.Exp)
                        t1 = tmp.tile([16, RPC], F32, tag="t1")
                        nc.vector.tensor_scalar_min(t1[:], texp[:], 1.0)
                        nc.vector.scalar_tensor_tensor(
                            contp[:, ch, :], nrm[:], 0.0, t1[:],
                            op0=OP.max, op1=OP.add)
                        # accumulate s1 / h'1 / d1 into ps1
                        nc.tensor.matmul(ps1[:], w1c[ch][:], contp[:, ch, :],
                                         start=(ch == 0), stop=(ch == 3))
                        if ch == 0:
                            nc.tensor.matmul(ps1[:], w1last[:], ones512[:],
                                             start=False, stop=False)

            # ---------------- Phase C: inter-layer + layer 1 ----------------
            with (
                tc.tile_pool(name="ld2", bufs=2) as ld2,
                tc.tile_pool(name="tp2", bufs=2, space="PSUM") as tp2,
                tc.tile_pool(name="pa1p", bufs=2, space="PSUM") as pa1p,
                tc.tile_pool(name="agg1", bufs=2, space="PSUM") as agg1,
                tc.tile_pool(name="e1pool", bufs=4) as e1pool,
                tc.tile_pool(name="rssp", bufs=2) as rssp,
                tc.tile_pool(name="otp", bufs=2) as otp,
            ):
                # s1 out the door first: evac -> DRAM -> AllGather
                nc.scalar.copy(s1s[:], ps1[64:65, :])
                nc.sync.dma_start(s1d[:], s1s[:])
                nc.gpsimd.collective_compute(
                    "AllGather", OP.bypass,
                    replica_groups=[list(range(NCORES))],
                    ins=[s1d.opt()], outs=[s1g.opt()])
                nc.sync.dma_start(s1raw[:], s1g[:])

                # local h'1 -> transposed bf16 stationary + d1 exps
                nc.scalar.copy(hp1s[0:32, :], ps1[0:32, :])
                nc.scalar.copy(hp1s[32:33, :], ps1[32:33, :])
                for q in range(4):
                    ptq = tp2.tile([128, 33], F32, tag="ptq")
                    nc.tensor.matmul(ptq[:],
                                     hp1s[:, q * 128:(q + 1) * 128],
                                     ident[0:33, 0:33], is_transpose=True)
                    nc.vector.tensor_copy(hpa1[:, q, 0:32], ptq[:, 0:32])
                    nc.scalar.activation(d1e[:, q:q + 1], ptq[:, 32:33],
                                         AF.Exp)
                    nc.scalar.activation(d1e2[:, q:q + 1], ptq[:, 32:33],
                                         AF.Exp, scale=0.2)
                nc.vector.memset(hpa1[:, :, 32:33], 1.0)
                nc.vector.memset(hpa1[:, :, 33:34], 0.0)

                # keep PE hot while the s1 AllGather is in flight
                wsrc2 = ld2.tile([128, 512], BF16, tag="wsrc2")
                nc.vector.memset(wsrc2[:], 0.5)
                wlhs2 = ld2.tile([128, 128], BF16, tag="wlhs2")
                nc.vector.memset(wlhs2[:], 0.25)
                wps2 = tp2.tile([128, 33], F32, tag="ptq")
                for r in range(80):
                    nc.tensor.matmul(wps2[:], wlhs2[:], wsrc2[:, 0:33],
                                     start=(r == 0), stop=(r == 79))

                # gathered s1 -> a1 = e^{0.8 s1} (bf16), bcast per dst chunk
                nc.scalar.activation(a1g[:], s1raw[:], AF.Exp, scale=0.8)

                for p in range(4):
                    for k in range(2):
                        ic = 2 * p + k
                        pa1 = pa1p.tile([128, RPC], F32, tag="pa1")
                        nc.tensor.matmul(pa1[:],
                                         sela[:, ic * 128:(ic + 1) * 128],
                                         a1g[:])
                        nc.vector.tensor_copy(a1t[:, ic, :], pa1[:])
                    pgs = [agg1.tile([33, RPC], F32, name=f"pg1_{p}_{k}",
                                     tag="pg1") for k in range(2)]
                    for jt in range(4):
                        e1 = e1pool.tile([128, 2 * RPC], BF16, tag="e1")
                        nc.vector.tensor_scalar(
                            e1[:].rearrange("p (a b) -> p a b", b=RPC),
                            a1t[:, 2 * p:2 * p + 2, :],
                            d1e[:, jt:jt + 1], d1e2[:, jt:jt + 1],
                            op0=OP.mult, op1=OP.max)
                        for k in range(2):
                            nc.tensor.matmul(
                                pgs[k][:], hpa1[:, jt, 0:33],
                                e1[:, k * RPC:(k + 1) * RPC],
                                start=(jt == 0), stop=(jt == 3))
                    for k in range(2):
                        rss = rssp.tile([33, RPC], BF16, tag="rss")
                        nc.scalar.copy(rss[0:32, :], pgs[k][0:32, :])
                        nc.scalar.copy(rss[32:33, :], pgs[k][32:33, :])
                        nc.sync.dma_start(rsin[2 * p + k, :, :], rss[:])

                # keep PE hot across the ReduceScatter
                wps3 = pa1p.tile([128, RPC], F32, tag="pa1")
                for r in range(40):
                    nc.tensor.matmul(wps3[:], wlhs2[:], wsrc2[:],
                                     start=(r == 0), stop=(r == 39))

                nc.gpsimd.collective_compute(
                    "ReduceScatter", OP.add,
                    replica_groups=[list(range(NCORES))],
                    ins=[rsin.opt()], outs=[rsout.opt()])
                nc.sync.dma_start(rso[:], rsout[:])

                # normalize + write out
                nc.scalar.copy(num32[:], rso[0:32, :])
                nc.scalar.copy(rcp1[:], rso[32:33, :])
                nc.vector.reciprocal_approx_fast(rcp1[:], rcp1[:])
                prb1 = agg1.tile([33, RPC], F32, tag="pg1")
                nc.tensor.matmul(prb1[0:32, :], ones32[:], rcp1[:])
                nc.vector.tensor_tensor(outv[:], num32[:], prb1[0:32, :],
                                        op=OP.mult)
                for q in range(4):
                    pt2 = tp2.tile([128, 33], F32, tag="ptq")
                    nc.tensor.matmul(pt2[:, 0:32],
                                     outv[:, q * 128:(q + 1) * 128],
                                     ident[0:32, 0:32], is_transpose=True)
                    ob = otp.tile([128, 32], F32, tag="ob")
                    nc.vector.tensor_copy(ob[:], pt2[:, 0:32])
                    nc.sync.dma_start(out_d[q * 128:(q + 1) * 128, :], ob[:])

    nc.compile()
    return nc


def _fold(inputs):
    """Host-side BN/bias/attention-projection folding (numpy, f64)."""
    f64 = np.float64
    x = np.asarray(inputs["x"], np.float32)
    xT = np.ascontiguousarray(x.T)              # [32, 4096]
    w0 = np.asarray(inputs["w0"], f64)          # [8, 32, 8]
    w1 = np.asarray(inputs["w1"], f64)          # [1, 64, 32]
    a_src0 = np.asarray(inputs["a_src0"], f64)[..., 0]   # [8, 8]
    a_dst0 = np.asarray(inputs["a_dst0"], f64)[..., 0]   # [8, 8]
    a_src1 = np.asarray(inputs["a_src1"], f64)[0, :, 0]  # [32]
    a_dst1 = np.asarray(inputs["a_dst1"], f64)[0, :, 0]  # [32]
    b0 = np.asarray(inputs["b0"], f64)          # [8]
    b1 = np.asarray(inputs["b1"], f64)          # [32]

    al0 = np.asarray(inputs["bn0_gamma"], f64) / np.sqrt(
        np.asarray(inputs["bn0_var"], f64) + BN_EPS)
    sh0 = np.asarray(inputs["bn0_beta"], f64) - \
        np.asarray(inputs["bn0_mean"], f64) * al0
    al1 = np.asarray(inputs["bn1_gamma"], f64) / np.sqrt(
        np.asarray(inputs["bn1_var"], f64) + BN_EPS)
    sh1 = np.asarray(inputs["bn1_beta"], f64) - \
        np.asarray(inputs["bn1_mean"], f64) * al1

    w0flat = (al0[None, :, None] * w0).transpose(1, 0, 2).reshape(32, 64)
    beta0h = np.einsum("i,hio->ho", sh0, w0)     # [8, 8]
    beta0 = (beta0h + b0[None, :]).reshape(64)
    as0 = al0[:, None] * np.einsum("hio,ho->ih", w0, a_src0)   # [32, 8]
    sb0 = np.einsum("ho,ho->h", beta0h, a_src0)
    ad0 = al0[:, None] * np.einsum("hio,ho->ih", w0, a_dst0)
    db0 = np.einsum("ho,ho->h", beta0h, a_dst0)

    w0all = np.zeros((33, 72), f64)
    w0all[0:32, 0:64] = w0flat
    w0all[32, 0:64] = beta0
    w0all[0:32, 64:72] = ad0
    w0all[32, 64:72] = db0
    w0s = np.zeros((33, 8), f64)
    w0s[0:32, :] = as0
    w0s[32, :] = sb0

    # layer 1 folds; input arrives as contp = elu(out0)+1
    w1m = w1[0]                                   # [64, 32]
    w1flat = al1[:, None] * w1m
    beta1 = sh1 @ w1m + b1
    as1 = al1 * (w1m @ a_src1)
    sb1 = (sh1 @ w1m) @ a_src1
    ad1 = al1 * (w1m @ a_dst1)
    db1 = (sh1 @ w1m) @ a_dst1

    w1ext = np.zeros((65, 65), f64)
    w1ext[0:64, 0:32] = w1flat
    w1ext[64, 0:32] = beta1 - w1flat.sum(axis=0)
    w1ext[0:64, 32] = ad1
    w1ext[64, 32] = db1 - ad1.sum()
    w1ext[0:64, 64] = as1
    w1ext[64, 64] = sb1 - as1.sum()

    sela = np.zeros((8, 8, 128), ml_dtypes.bfloat16)
    for h in range(8):
        sela[h, h, :] = 1.0
    s2sel = np.zeros((2, 16), np.float32)
    for p in range(2):
        s2sel[p, p * 8:(p + 1) * 8] = 1.0

    return {
        "xT": xT,
        "w0all": w0all.astype(np.float32),
        "w0s": w0s.astype(np.float32),
        "w1ext": w1ext.astype(np.float32),
        "sela": sela.reshape(8, 8 * 128),
        "s2sel": s2sel,
    }


def kernel(**inputs) -> np.ndarray:
    if "nc" not in _CACHE:
        _CACHE["nc"] = _build()
    nc = _CACHE["nc"]

    shared = _fold(inputs)
    xT = shared["xT"]
    in_maps = []
    for c in range(NCORES):
        m = dict(shared)
        m["xsT"] = np.ascontiguousarray(xT[:, c * RPC:(c + 1) * RPC])
        in_maps.append(m)

    res = run_bass_kernel_spmd(nc, in_maps, list(range(NCORES)))
    out = np.concatenate([res.results[c]["out"] for c in range(NCORES)],
                         axis=0)
    return out.astype(np.float32)
